# revision 5
# baseline (speedup 1.0000x reference)
"""Trainium2 Bass kernel for CovarianceSimilarity (v2).

scores[n,w] = sum_k conv_w[k]*leaky_relu(diag(Qn^T cov_w Qn)[k]) + conv_b

Empirical cost model of the axon-tunneled PJRT path (from probes):
  - fixed dispatch floor ~175ms per invocation (unavoidable)
  - per-run input cost ~0.75ms/MB of per-core staged inputs above ~4MB
    -> inputs sent as bf16, no transposed-query copy, support sharded
       (17MB/core vs baseline's 55MB/core)
  - a single collective is ~free; sequences of them are not
    -> one merged AllReduce (A moments + sample sums in one buffer)
  - gpsimd ops carry a large fixed per-run cost -> none used
  - PE/DVE/Scalar compute streams are hidden under the floor
    -> full V (no triangle masking), bf16 matmuls

Strategy (8 cores, data-parallel over NQ):
  - Phase A: each core computes partial moments A_w = S^T S and sample
    sums s_w over its 1/8 chunk of the samples; ONE AllReduce combines
    both; V_w = A_w/(N-1) - outer(s',s') stored bf16.
  - Phase B per query: one DMA -> [128, 5*1024] bf16 tile; per-channel
    inv-norms (Square accum + Sqrt + reciprocal + scaled Copy);
    Yt[b,k] = sum_c V[c,b] Qn[c,k] PSUM-accumulated over c-blocks;
    Z = Yt o Qn on DVE; F row via ones-matmuls; LeakyReLU + conv dot.
"""

import numpy as np
from contextlib import ExitStack

import concourse.bass as bass
import concourse.tile as tile
from concourse import bacc, mybir
from concourse.bass_utils import run_bass_kernel_spmd

# ---- problem constants (hardcoded per spec) ----
NQ, C, D = 75, 640, 1024
WAY, SHOT = 5, 5
NTOT = SHOT * D            # 5120 samples per way
NCORES = 8
QPC = 10                   # queries per core (NQ padded to 80)
NCHUNK = NTOT // NCORES    # 640 samples per core per way
NBL = NCHUNK // 128        # 5 sample blocks per core per way
EPS = 1e-8
NEG_SLOPE = 0.2
CB = C // 128              # 5 c-blocks
SCALE = 1.0 / (NTOT - 1 + EPS)
ACOLS = WAY * CB * C       # 16000 columns of packed partial A
F32 = mybir.dt.float32
BF16 = mybir.dt.bfloat16
FP8 = mybir.dt.float8e3

_CACHE = {}


def _build():
    nc = bacc.Bacc("TRN2", target_bir_lowering=False, debug=False,
                   num_devices=NCORES)
    qs_ap = nc.dram_tensor("qs", [QPC, CB, 128, D], BF16,
                           kind="ExternalInput").ap()
    st_ap = nc.dram_tensor("st", [WAY, NBL, 128, C], FP8,
                           kind="ExternalInput").ap()
    cwb_ap = nc.dram_tensor("cwb", [1, D + 1], F32,
                            kind="ExternalInput").ap()
    out_ap = nc.dram_tensor("out", [1, QPC * WAY], F32,
                            kind="ExternalOutput").ap()

    with tile.TileContext(nc) as tc:
        with ExitStack() as ctx:
            _body(nc, tc, ctx, qs_ap, st_ap, cwb_ap, out_ap)
    nc.compile()
    return nc


def _body(nc, tc, ctx, qs_ap, st_ap, cwb_ap, out_ap):
    # ---------------- persistent pools ----------------
    vpool = ctx.enter_context(tc.tile_pool(name="v", bufs=1))
    const = ctx.enter_context(tc.tile_pool(name="const", bufs=1))
    rpool = ctx.enter_context(tc.tile_pool(name="r", bufs=1))

    ones32 = const.tile([128, 1], F32, tag="ones32")
    nc.vector.memset(ones32[:], 1.0)
    onesb = const.tile([128, 1], BF16, tag="onesb")
    nc.vector.tensor_copy(onesb[:], ones32[:])
    cwb_sb = const.tile([1, D + 1], F32, tag="cwb")
    nc.sync.dma_start(cwb_sb[:], cwb_ap[:])
    cw_sb = cwb_sb[0:1, 0:D]
    cb_sb = cwb_sb[0:1, D:D + 1]

    # V tiles (bf16): vt[w][m] = [128, C], m-th 128-row block of V_w
    vt = [[vpool.tile([128, C], BF16, tag=f"v{w}_{m}", name=f"v{w}_{m}")
           for m in range(CB)] for w in range(WAY)]

    # scores accumulator row
    sc_acc = rpool.tile([1, QPC * WAY], F32, tag="scacc")

    # ---------------- phase A: sharded covariance + one AllReduce ----------
    with tc.tile_pool(name="covdram", bufs=1, space="DRAM") as cov_dram, \
         ExitStack() as actx:
        a_in = cov_dram.tile([128, ACOLS + WAY * C], F32)       # 9.8 MB
        a_out = cov_dram.tile([128, ACOLS + WAY * C], F32,
                              addr_space="Shared")

        stp = actx.enter_context(tc.tile_pool(name="st", bufs=2))
        st8p = actx.enter_context(tc.tile_pool(name="st8", bufs=2))
        a5p = actx.enter_context(tc.tile_pool(name="a5", bufs=2))
        srowp = actx.enter_context(tc.tile_pool(name="srow", bufs=1))
        with tc.tile_pool(name="apsum", bufs=2, space="PSUM") as apsum, \
             tc.tile_pool(name="spsum", bufs=1, space="PSUM") as spsum, \
             tc.tile_pool(name="opsum", bufs=1, space="PSUM") as opsum:
            s_sb = srowp.tile([1, WAY * C], F32, tag="ssb")
            # zero the slack partitions of the s segment so the summed
            # buffer stays finite (partition 0 carries the real s rows)
            zfill = srowp.tile([128, WAY * C], F32, tag="zfill")
            nc.vector.memset(zfill[:], 0.0)
            nc.sync.dma_start(a_in[1:128, ACOLS:ACOLS + WAY * C],
                              zfill[0:127, :])
            for w in range(WAY):
                st8 = st8p.tile([128, NBL * C], FP8, tag="st8")
                nc.sync.dma_start(st8[:], st_ap[w].transpose([1, 0, 2]))
                st_t = stp.tile([128, NBL * C], BF16, tag="stc")
                nc.vector.tensor_copy(st_t[:], st8[:])
                # s_w partial row via ones-matmuls
                s_ps = spsum.tile([1, C], F32, tag="sps")
                for a in range(NBL):
                    for lo, hi in ((0, 512), (512, C)):
                        nc.tensor.matmul(s_ps[:, lo:hi], onesb[:],
                                         st_t[:, a * C + lo:a * C + hi],
                                         start=(a == 0), stop=(a == NBL - 1),
                                         skip_group_check=True)
                nc.vector.tensor_copy(s_sb[0:1, w * C:(w + 1) * C], s_ps[:])
                # A_w partial block-rows
                a5 = a5p.tile([128, CB * C], F32, tag="a5")
                for m in range(CB):
                    a_ps = apsum.tile([128, C], F32, tag="aps")
                    for a in range(NBL):
                        lhsT = st_t[:, a * C + m * 128:a * C + (m + 1) * 128]
                        for lo, hi in ((0, 512), (512, C)):
                            nc.tensor.matmul(a_ps[:, lo:hi], lhsT,
                                             st_t[:, a * C + lo:a * C + hi],
                                             start=(a == 0),
                                             stop=(a == NBL - 1),
                                             skip_group_check=True)
                    nc.vector.tensor_copy(a5[:, m * C:(m + 1) * C], a_ps[:])
                nc.sync.dma_start(
                    a_in[:, w * CB * C:(w + 1) * CB * C], a5[:])
            nc.sync.dma_start(a_in[0:1, ACOLS:ACOLS + WAY * C], s_sb[:])

            nc.gpsimd.collective_compute(
                "AllReduce", mybir.AluOpType.add,
                replica_groups=[list(range(NCORES))],
                ins=[a_in.opt()], outs=[a_out.opt()])

            # ---- V prep ----
            alodp = actx.enter_context(tc.tile_pool(name="alod", bufs=1))
            a_all = alodp.tile([128, ACOLS], F32, tag="aall")
            nc.sync.dma_start(a_all[:], a_out[:, 0:ACOLS])
            s_all = srowp.tile([1, WAY * C], F32, tag="sall")
            nc.sync.dma_start(s_all[:], a_out[0:1, ACOLS:ACOLS + WAY * C])
            sp_all = srowp.tile([1, WAY * C], F32, tag="spall")
            # s' = s * sqrt(SCALE / NTOT); outer(s',s') = SCALE*s s^T/NTOT
            nc.vector.tensor_scalar_mul(sp_all[:], s_all[:],
                                        float(np.sqrt(SCALE / NTOT)))
            for w in range(WAY):
                for m in range(CB):
                    o_ps = opsum.tile([128, C], F32, tag="ops")
                    for lo, hi in ((0, 512), (512, C)):
                        nc.tensor.matmul(
                            o_ps[:, lo:hi],
                            sp_all[0:1, w * C + m * 128:w * C + (m + 1) * 128],
                            sp_all[0:1, w * C + lo:w * C + hi],
                            start=True, stop=True, skip_group_check=True)
                    # V = SCALE*A - outer  (rounded to bf16 on write)
                    nc.vector.scalar_tensor_tensor(
                        vt[w][m][:], a_all[:, (w * CB + m) * C:
                                           (w * CB + m + 1) * C],
                        SCALE, o_ps[:],
                        op0=mybir.AluOpType.mult,
                        op1=mybir.AluOpType.subtract)

    # ---------------- phase B: queries ----------------
    qraw = ctx.enter_context(tc.tile_pool(name="qraw", bufs=3))
    qnp = ctx.enter_context(tc.tile_pool(name="qn", bufs=3))
    nrm = ctx.enter_context(tc.tile_pool(name="nrm", bufs=3))
    zpool = ctx.enter_context(tc.tile_pool(name="z", bufs=4))
    xpool = ctx.enter_context(tc.tile_pool(name="x", bufs=2))
    scratch = ctx.enter_context(tc.tile_pool(name="scr", bufs=3))

    with tc.tile_pool(name="ypsum", bufs=3, space="PSUM") as ypsum, \
         tc.tile_pool(name="fpsum", bufs=1, space="PSUM") as fpsum:
        for q in range(QPC):
            raw = qraw.tile([128, CB * D], BF16, tag="qraw")
            nc.sync.dma_start(raw[:], qs_ap[q].transpose([1, 0, 2]))
            # per-channel inv norms; qn = raw * inv (bf16)
            qn = qnp.tile([128, CB * D], BF16, tag="qn")
            invs = []
            for cb in range(CB):
                sq = scratch.tile([128, D], F32, tag="sq")
                ssq = nrm.tile([128, 1], F32, tag=f"ssq{cb}")
                nc.scalar.activation(sq[:], raw[:, cb * D:(cb + 1) * D],
                                     mybir.ActivationFunctionType.Square,
                                     accum_out=ssq[:])
                nrm_t = nrm.tile([128, 1], F32, tag=f"nrm{cb}")
                nc.scalar.activation(nrm_t[:], ssq[:],
                                     mybir.ActivationFunctionType.Sqrt)
                inv = nrm.tile([128, 1], F32, tag=f"inv{cb}")
                nc.vector.reciprocal(inv[:], nrm_t[:])
                invs.append(inv)
            for cb in range(CB):
                nc.scalar.activation(qn[:, cb * D:(cb + 1) * D],
                                     raw[:, cb * D:(cb + 1) * D],
                                     mybir.ActivationFunctionType.Copy,
                                     scale=invs[cb])
            for w in range(WAY):
                f_ps = fpsum.tile([1, D], F32, tag="fps")
                for bb in range(CB):
                    y_ps = ypsum.tile([128, D], F32, tag="yps")
                    for p in range(CB):
                        lhsT = vt[w][p][:, bb * 128:(bb + 1) * 128]
                        for lo, hi in ((0, 512), (512, D)):
                            nc.tensor.matmul(y_ps[:, lo:hi], lhsT,
                                             qn[:, p * D + lo:p * D + hi],
                                             start=(p == 0), stop=(p == CB - 1),
                                             skip_group_check=True)
                    z_t = zpool.tile([128, D], BF16, tag="z")
                    nc.vector.scalar_tensor_tensor(
                        z_t[:], y_ps[:], 1.0, qn[:, bb * D:(bb + 1) * D],
                        op0=mybir.AluOpType.mult, op1=mybir.AluOpType.mult)
                    for lo, hi in ((0, 512), (512, D)):
                        nc.tensor.matmul(f_ps[:, lo:hi], onesb[:],
                                         z_t[:, lo:hi],
                                         start=(bb == 0), stop=(bb == CB - 1),
                                         skip_group_check=True)
                x_t = xpool.tile([1, D], F32, tag="xt")
                nc.scalar.activation(x_t[:], f_ps[:],
                                     mybir.ActivationFunctionType.Lrelu,
                                     alpha=NEG_SLOPE)
                cw_scr = scratch.tile([1, D], F32, tag="cwscr")
                nc.vector.scalar_tensor_tensor(
                    cw_scr[:], x_t[:], 1.0, cw_sb,
                    op0=mybir.AluOpType.mult, op1=mybir.AluOpType.mult,
                    accum_out=sc_acc[0:1, q * WAY + w:q * WAY + w + 1])

    # ---------------- final: add bias, store ----------------
    with tc.tile_pool(name="osb", bufs=1) as osb:
        sc_sb = osb.tile([1, QPC * WAY], F32, tag="scsb")
        nc.vector.tensor_scalar_add(sc_sb[:], sc_acc[:], cb_sb)
        nc.sync.dma_start(out_ap[:], sc_sb[:])


def _get_nc():
    if "nc" not in _CACHE:
        _CACHE["nc"] = _build()
    return _CACHE["nc"]


def _to_bf16(x):
    import ml_dtypes
    return np.asarray(x, dtype=np.float32).astype(ml_dtypes.bfloat16)


def _to_fp8(x):
    import ml_dtypes
    return np.asarray(x, dtype=np.float32).astype(ml_dtypes.float8_e3m4)


def _host_prep(query, support, conv_w, conv_b):
    q = np.ascontiguousarray(query.reshape(NQ, C, D), dtype=np.float32)
    pad = NCORES * QPC - NQ
    qpad = np.concatenate([q, np.broadcast_to(q[0:1], (pad, C, D))], axis=0)
    qpad = _to_bf16(qpad.reshape(NCORES * QPC, CB, 128, D))
    # [Way, n, C] with n = (shot, h, w); shard n across cores
    st_full = _to_fp8(np.ascontiguousarray(
        support.transpose(0, 1, 3, 4, 2).reshape(WAY, NTOT, C),
        dtype=np.float32))
    cwb = np.concatenate([
        np.asarray(conv_w, dtype=np.float32).reshape(1, D),
        np.asarray(conv_b, dtype=np.float32).reshape(1, 1)], axis=1)
    in_maps = []
    for c in range(NCORES):
        qs = np.ascontiguousarray(qpad[c * QPC:(c + 1) * QPC])
        st = np.ascontiguousarray(
            st_full[:, c * NCHUNK:(c + 1) * NCHUNK].reshape(
                WAY, NBL, 128, C))
        in_maps.append({"qs": qs, "st": st, "cwb": cwb})
    return in_maps


def kernel(query, support, conv_w, conv_b):
    in_maps = _host_prep(np.asarray(query), np.asarray(support),
                         np.asarray(conv_w), np.asarray(conv_b))
    nc = _get_nc()
    res = run_bass_kernel_spmd(nc, in_maps, core_ids=list(range(NCORES)))
    scores = np.concatenate(
        [res.results[c]["out"].reshape(QPC, WAY) for c in range(NCORES)],
        axis=0)[:NQ]
    return np.ascontiguousarray(scores, dtype=np.float32)


if __name__ == "__main__":
    import reference
    inputs = reference.setup_inputs()
    exp = np.asarray(reference.reference(**inputs))
    got = kernel(**{k: np.asarray(v) for k, v in inputs.items()})
    rel = np.abs(got - exp).max() / np.abs(exp).max()
    print(f"Relative error: {rel:.3e}")
